# revision 13
# baseline (speedup 1.0000x reference)
"""Sparse-attention Trainium2 kernel v2 (nn_Attention_44341242364527).

Head-tensor-parallel over 8 NeuronCores (2 heads/core).  Dense scatter
reformulation of the sparse gather (WT = scatter of exp(geo_bias), causal
mask folded in), as in v1, with these changes:

- Per-tile softmax-denominator matmuls are gone: at-tiles accumulate into a
  per-(j,h) bf16 SBUF tile on DVE (2x mode; <=16-deep sum, ~0.4% rounding),
  then ONE ones-vector matmul per (j,h) gives Z (512-col stream) and a
  rank-1 matmul broadcasts the reciprocal.  Saves ~13us of PE streaming vs
  per-tile Z.  (GPSIMD partition_all_reduce was tried and is far slower on
  real HW than its cost model; GPSIMD ucode library swaps cost ~tens of us
  per use - keep Pool to memset only.  DVE cannot read two PSUM operands in
  one op, so the reciprocal broadcast is staged through SBUF via ACT.)
- bf16 for at/Vsb/AOT/wo (DVE 2x mode on the exp*wt multiply, FWL on
  matmul weight loads).  QT/KT stay f32r so exp() input error stays tiny.
- Dedicated PSUM pools per stage (proj/ST/AV/Y, 2 banks each) instead of a
  shared pool; psY=1 in v1 serialized the output projection tail.
- Batched DMA: hs chunk is one 2MB transfer, contiguous per partition via
  j-major host layout [P, NJ, NK, SC]; QKV weights one transfer each; y per
  128-row block; next chunk's hs trigger is emitted before this chunk's y
  triggers so the in-order SP queue can't stall the prefetch.
- Engine balance: exp + V-proj copies + half the y copies on ACT, the rest
  (QT/KT copies, multiplies, z-adds, reciprocal, normalize) on DVE.
"""

import math
import sys

sys.path.insert(0, "/opt/trn_rl_repo")

import numpy as np

B, S, H, D, KS = 1, 2048, 16, 128, 64
HID = H * D
NCORES = 8
HPC = H // NCORES          # heads per core
CPC = HPC * D              # output channels per core
P = 128                    # partitions
SC = 512                   # s-chunk (PSUM bank width in f32)
NJ = S // SC               # 4 s-chunks
NT = S // P                # 16 s'-tiles
NK = HID // P              # 16 contraction chunks

_CACHE = {}

CFG = dict(psP=2, psS=2, psA=2, psY=2, lag=4, atp=10, wtp=8)


def _np_bf16():
    import ml_dtypes

    return np.dtype(ml_dtypes.bfloat16)


def _build_nc(reps=1, cfg=None):
    import concourse.tile as tile
    from concourse import bacc, bass_isa, mybir

    c = dict(CFG)
    if cfg:
        c.update(cfg)

    F32 = mybir.dt.float32
    F32R = mybir.dt.float32r
    BF16 = mybir.dt.bfloat16
    EXP = mybir.ActivationFunctionType.Exp
    MULT = mybir.AluOpType.mult
    ADD = mybir.AluOpType.add
    RADD = bass_isa.ReduceOp.add

    nc = bacc.Bacc("TRN2", target_bir_lowering=False, debug=False,
                   num_devices=NCORES)

    hsr = nc.dram_tensor("hsr", [P, NJ, NK, SC], BF16, kind="ExternalInput")
    wqr = nc.dram_tensor("wqr", [P, NK, CPC], BF16, kind="ExternalInput")
    wkr = nc.dram_tensor("wkr", [P, NK, CPC], BF16, kind="ExternalInput")
    wvr = nc.dram_tensor("wvr", [P, NK, CPC], BF16, kind="ExternalInput")
    wor = nc.dram_tensor("wor", [CPC, HID], BF16, kind="ExternalInput")
    wt = nc.dram_tensor("wt", [HPC, S, S], BF16, kind="ExternalInput")
    y = nc.dram_tensor("y", [S, HID], BF16, kind="ExternalOutput")

    inv_sqrt_d = 1.0 / math.sqrt(D)

    def mm(out, lhsT, rhs, **kw):
        nc.tensor.matmul(out, lhsT, rhs, **kw)

    with tile.TileContext(nc) as tc, \
            nc.allow_low_precision(reason="bf16/f32r matmul operands; PSUM f32"):
        with tc.tile_pool(name="persist", bufs=1) as persist:
            # Per-chunk / per-tile staging tiles so dependency tracking never
            # serializes a chunk's writes against the previous chunk's reads
            # (whole-tile WAR).  KT/Vsb persist across chunks of a rep, so
            # they are double-buffered by rep parity.
            KTc = [[[persist.tile([P, SC], F32R, tag=f"kt{h}_{jj}_{pp}",
                                  name=f"kt{h}_{jj}_{pp}")
                     for jj in range(NJ)] for h in range(HPC)]
                   for pp in range(2)]
            Vsb2 = [[persist.tile([P, CPC], BF16, tag=f"v{t}_{pp}",
                                  name=f"vres{t}_{pp}")
                     for t in range(NT)] for pp in range(2)]
            ones_col = persist.tile([P, 1], BF16, tag="ones_col",
                                    name="ones_col")
            ones_row = persist.tile([1, P], F32R, tag="ones_row",
                                    name="ones_row")
            ones_f32 = persist.tile([P, 1], F32, tag="ones_f32",
                                    name="ones_f32")
            onesr_f32 = persist.tile([1, P], F32, tag="onesr_f32",
                                     name="onesr_f32")
            nc.gpsimd.memset(ones_f32[:], 1.0)
            nc.gpsimd.memset(onesr_f32[:], 1.0)
            nc.vector.tensor_copy(ones_col[:], ones_f32[:])
            nc.vector.tensor_copy(ones_row[:], onesr_f32[:])

            with tc.tile_pool(name="wpool", bufs=1) as wpool, \
                 tc.tile_pool(name="hpool", bufs=2) as hpool, \
                 tc.tile_pool(name="qpool", bufs=6) as qpool, \
                 tc.tile_pool(name="aotp", bufs=6) as aotp, \
                 tc.tile_pool(name="wtp", bufs=c["wtp"]) as wtp, \
                 tc.tile_pool(name="atp", bufs=c["atp"]) as atp, \
                 tc.tile_pool(name="zpool", bufs=6) as zpool, \
                 tc.tile_pool(name="zbp", bufs=4) as zbp, \
                 tc.tile_pool(name="rbp", bufs=4) as rbp, \
                 tc.tile_pool(name="ypool", bufs=3) as ypool, \
                 tc.tile_pool(name="psP", bufs=c["psP"], space="PSUM") as psP, \
                 tc.tile_pool(name="psS", bufs=c["psS"], space="PSUM") as psS, \
                 tc.tile_pool(name="psA", bufs=c["psA"], space="PSUM") as psA, \
                 tc.tile_pool(name="psY", bufs=c["psY"], space="PSUM") as psY:
                wq_sb = wk_sb = wv_sb = None
                wo_sb = []

                def load_hs(jj):
                    # j-major host layout: the chunk is contiguous per
                    # partition (16KB) -> 128 large DMA descriptors
                    t = hpool.tile([P, NK * SC], BF16, tag="hs", name="hs")
                    nc.sync.dma_start(t[:], hsr[:, jj, :, :])
                    return t

                seq = [(r, j) for r in range(reps) for j in range(NJ)]
                next_hs = load_hs(0)
                for step, (_rep, j) in enumerate(seq):
                    if True:
                        KT = KTc[_rep % 2]
                        Vsb = Vsb2[_rep % 2]
                        # hs for this chunk was prefetched a chunk ago; kick
                        # off the next chunk's load before anything else so
                        # its trigger sits ahead of y DMAs in the SP queue
                        hs_j = next_hs
                        if step + 1 < len(seq):
                            next_hs = load_hs(seq[step + 1][1])
                        if _rep == 0 and j == 0:
                            wq_sb = wpool.tile([P, NK * CPC], BF16, tag="wq",
                                               name="wq")
                            wk_sb = wpool.tile([P, NK * CPC], BF16, tag="wk",
                                               name="wk")
                            wv_sb = wpool.tile([P, NK * CPC], BF16, tag="wv",
                                               name="wv")
                            nc.sync.dma_start(wq_sb[:], wqr[:, :, :])
                            nc.sync.dma_start(wk_sb[:], wkr[:, :, :])
                            nc.sync.dma_start(wv_sb[:], wvr[:, :, :])
                            for h in range(HPC):
                                t_ = wpool.tile([P, HID], BF16, tag=f"wo{h}",
                                                name=f"wo{h}")
                                nc.sync.dma_start(
                                    t_[:], wor[h * P:(h + 1) * P, :])
                                wo_sb.append(t_)

                        # -- QKV projection for chunk j --
                        # nk_eff < NK is a TIMING PROBE ONLY (wrong numerics)
                        nk_eff = c.get("nk_eff", NK)
                        QTj = [None] * HPC
                        for h in range(HPC):
                            for w_sb, is_q in ((wq_sb, True), (wk_sb, False)):
                                pp = psP.tile([P, SC], F32, tag="p",
                                              name="ps_proj")
                                for k in range(nk_eff):
                                    mm(pp[:],
                                       w_sb[:, k * CPC + h * D:
                                            k * CPC + (h + 1) * D],
                                       hs_j[:, k * SC:(k + 1) * SC],
                                       start=(k == 0), stop=(k == nk_eff - 1))
                                if is_q:
                                    QTj[h] = qpool.tile([P, SC], F32R,
                                                        tag="q", name="qtj")
                                    nc.vector.tensor_copy(QTj[h][:], pp[:])
                                else:
                                    nc.vector.tensor_copy(KT[h][j][:], pp[:])
                        for si in range(SC // P):
                            vp = psP.tile([P, CPC], F32, tag="p",
                                          name="ps_projv")
                            for k in range(nk_eff):
                                mm(vp[:],
                                   hs_j[:, k * SC + si * P:
                                        k * SC + (si + 1) * P],
                                   wv_sb[:, k * CPC:(k + 1) * CPC],
                                   start=(k == 0), stop=(k == nk_eff - 1))
                            nc.scalar.copy(Vsb[4 * j + si][:], vp[:])

                        # -- attention for chunk j --
                        tmax = min(4 * j + 3, NT - 1)
                        aop = [psA.tile([P, SC], F32, tag="ao", name=f"ao{h}")
                               for h in range(HPC)]
                        # bf16 zacc: adds run in DVE 2x mode; <=16-deep
                        # accumulation keeps Z rounding error ~0.4%
                        zacc = [zpool.tile([P, SC], BF16, tag="z",
                                           name=f"z{h}")
                                for h in range(HPC)]
                        items = [(t, h) for t in range(tmax + 1)
                                 for h in range(HPC)]
                        pend = []

                        def drain_one():
                            t_, h_, at_, o_, w_ = pend.pop(0)
                            mm(aop[h_][:, o_:SC],
                               Vsb[t_][:, h_ * D:(h_ + 1) * D],
                               at_[:, :w_],
                               start=(t_ == 0), stop=(t_ == tmax))

                        for t, h in items:
                            o = max(0, t * P - j * SC)
                            w = SC - o
                            stp = psS.tile([P, SC], F32, tag="st", name="st")
                            mm(stp[:, :w],
                               KT[h][t // 4][:, (t % 4) * P:(t % 4 + 1) * P],
                               QTj[h][:, o:SC],
                               start=True, stop=True)
                            at = atp.tile([P, SC], BF16, tag="at", name="at")
                            nc.scalar.activation(at[:, :w], stp[:, :w], EXP,
                                                 scale=inv_sqrt_d)
                            wt_sb = wtp.tile([P, SC], BF16, tag="wt",
                                             name="wt")
                            nc.sync.dma_start(
                                wt_sb[:, :w],
                                wt[h, t * P:(t + 1) * P,
                                   j * SC + o:(j + 1) * SC])
                            nc.vector.tensor_mul(at[:, :w], at[:, :w],
                                                 wt_sb[:, :w])
                            # Z accumulation on DVE (GPSIMD must stay
                            # single-library: only partition_all_reduce)
                            if t == 0:
                                nc.vector.tensor_copy(zacc[h][:], at[:])
                            else:
                                nc.vector.tensor_tensor(zacc[h][:, o:SC],
                                                        zacc[h][:, o:SC],
                                                        at[:, :w], ADD)
                            pend.append((t, h, at, o, w))
                            if len(pend) >= c["lag"]:
                                drain_one()
                        while pend:
                            drain_one()

                        aot_j = [None] * HPC
                        for h in range(HPC):
                            # Z = ones^T @ zacc (single 512-col stream), then
                            # reciprocal broadcast back via a rank-1 matmul.
                            # All on PE/DVE/ACT: no GPSIMD on the critical
                            # path (its software ucode is slow on HW).
                            zp = psY.tile([1, SC], F32, tag="yy", name="zp")
                            mm(zp[:], ones_col[:], zacc[h][:],
                               start=True, stop=True)
                            r_sb = zbp.tile([1, SC], F32R, tag="r", name="r")
                            nc.vector.reciprocal(r_sb[:], zp[:])
                            rb_ps = psY.tile([P, SC], F32, tag="yy",
                                             name="rb_ps")
                            mm(rb_ps[:], ones_row[:], r_sb[:],
                               start=True, stop=True)
                            rbs = rbp.tile([P, SC], F32, tag="rb",
                                           name="rbs")
                            nc.scalar.copy(rbs[:], rb_ps[:])
                            aot_j[h] = aotp.tile([P, SC], BF16, tag="aot",
                                                 name="aot")
                            nc.vector.tensor_tensor(
                                aot_j[h][:], aop[h][:], rbs[:], MULT)

                        # -- output projection for s-tiles of chunk j --
                        for m in range(4 * j, 4 * j + 4):
                            ysb = ypool.tile([P, HID], BF16, tag="y",
                                             name="ysb")
                            for n in range(NJ):
                                yps = psY.tile([P, SC], F32, tag="yy",
                                               name="ps_y")
                                for h in range(HPC):
                                    mm(yps[:],
                                       aot_j[h][:, (m - 4 * j) * P:
                                                (m - 4 * j + 1) * P],
                                       wo_sb[h][:, n * SC:(n + 1) * SC],
                                       start=(h == 0), stop=(h == HPC - 1))
                                # alternate the PSUM->bf16 copies between ACT
                                # and DVE so neither engine's FIFO backs up
                                if n % 2 == 0:
                                    nc.scalar.copy(
                                        ysb[:, n * SC:(n + 1) * SC], yps[:])
                                else:
                                    nc.vector.tensor_copy(
                                        ysb[:, n * SC:(n + 1) * SC], yps[:])
                            nc.sync.dma_start(y[m * P:(m + 1) * P, :],
                                              ysb[:])

    nc.compile()
    return nc


def _get_nc():
    if "nc" not in _CACHE:
        _CACHE["nc"] = _build_nc()
    return _CACHE["nc"]


def make_in_maps(hidden_states, idx, valid, geo_bias, Wq, Wk, Wv, Wo):
    """Host-side sharding/layout prep: returns the 8 per-core input maps."""
    bf16 = _np_bf16()
    hs = np.ascontiguousarray(np.asarray(hidden_states, np.float32)[0])
    idx = np.asarray(idx).astype(np.int64)
    valid = np.asarray(valid).astype(bool)

    # hsT [HID, S] -> [P, NJ, NK, SC]  ((p, j, k, s) = hsT[k*P+p, j*SC+s])
    hsT = np.ascontiguousarray(hs.T)
    hsr = np.ascontiguousarray(
        hsT.reshape(NK, P, NJ, SC).transpose(1, 2, 0, 3)).astype(bf16)

    srange = np.arange(S)
    cmask = ((idx <= srange[:, None]) & valid).ravel()
    flat = (idx * S + srange[:, None]).ravel()[cmask]
    eg = np.exp(np.asarray(geo_bias, np.float64))          # [H, S, K]

    def shard_w(Wfull, sl):
        # W[sl].T [HID, CPC] -> [P, NK, CPC]
        wT = np.ascontiguousarray(np.asarray(Wfull)[sl].T)
        return np.ascontiguousarray(
            wT.reshape(NK, P, CPC).transpose(1, 0, 2)).astype(bf16)

    in_maps = []
    for cix in range(NCORES):
        h0 = HPC * cix
        sl = slice(h0 * D, (h0 + HPC) * D)
        wt_c = np.empty((HPC, S, S), bf16)
        for hh in range(HPC):
            wt_c[hh] = (np.bincount(flat,
                                    weights=eg[h0 + hh].ravel()[cmask],
                                    minlength=S * S)
                        .reshape(S, S).astype(bf16))
        in_maps.append({
            "hsr": hsr,
            "wqr": shard_w(Wq, sl),
            "wkr": shard_w(Wk, sl),
            "wvr": shard_w(Wv, sl),
            "wor": np.ascontiguousarray(
                np.asarray(Wo)[:, sl].T).astype(bf16),
            "wt": wt_c,
        })
    return in_maps


def kernel(hidden_states, idx, valid, geo_bias, Wq, Wk, Wv, Wo, bo):
    from concourse import bass_utils

    nc = _get_nc()
    in_maps = make_in_maps(hidden_states, idx, valid, geo_bias, Wq, Wk, Wv,
                           Wo)
    res = bass_utils.run_bass_kernel_spmd(nc, in_maps,
                                          core_ids=list(range(NCORES)))
    out = np.zeros((S, HID), np.float32)
    for r in res.results:
        out += r["y"].astype(np.float32)
    out += np.asarray(bo, np.float32)
    return out.reshape(B, S, HID)
